# revision 1
# baseline (speedup 1.0000x reference)
"""Trainium2 Bass kernel for prefix-LM CausalSelfAttention.

Problem: B=2, T=2048, C=2048, H=16 heads (hd=128), prefix-LM mask
(bidirectional over first half, causal after), RoPE on q/k.

Sharding over 8 cores: data-parallel on batch (2) x tensor-parallel on
heads (4 heads per core). Each core computes a partial output projection
(its heads' contribution); partials are summed on host.

Per-core dataflow (all matmuls in float32r: full PE rate, ~1e-4 rel err):
  1. qT/kT = W^T @ x^T    [hd*4, T] "transposed" layout (head-major tiles)
  2. RoPE via pair-swap permutation matmul + DVE combine with cos/sin planes
  3. v = x @ Wv           [T, hd*4] natural layout
  4. Per head, per 512-wide query chunk I, over unmasked 128-key tiles J:
       S'[J] = k_rope[:,J]^T-tile x q_rope[:,I]      (scores transposed, [j,i])
       P'[J] = exp(S' * 1/sqrt(hd))                  (ACT, PSUM->SBUF, f32r)
       mask-multiply for diagonal-crossing tiles only (4 static patterns)
       y_psum  += v[J,h]^T-as-lhsT x P'[J]           (PV, out y^T [hd, i])
       d_psum  += ones^T x P'[J]                     (softmax denominator)
     y^T[:, I] = y_psum * broadcast(1/d)             (normalize, f32r)
  5. partial_out = y^T-as-lhsT x Wp  accumulated over the 4 heads.

Fully-masked key tiles are skipped (structural sparsity: 44/64 tiles/head).
"""
import math

import numpy as np

N_HEAD = 16
B = 2
T = 2048
C = 2048
HD = 128
HPC = 4          # heads per core
CL = HPC * HD    # local C = 512
TC = 512         # chunk width (matmul moving free dim / psum bank)
NT = T // TC     # 4 chunks
KT = C // 128    # 16 contraction tiles over C
TT = T // 128    # 16 T tiles
SCALE = 1.0 / math.sqrt(HD)

# Per query-chunk I: list of (J, mask_idx) key tiles to compute.
# mask_idx is None for fully-allowed tiles, else 0..3 selecting the
# static diagonal pattern mask[d][jj, ii] = (ii >= jj + 128*d).
_JLISTS = {
    0: [(j, None) for j in range(8)],
    1: [(j, None) for j in range(8)],
    2: [(j, None) for j in range(8)] + [(8 + d, d) for d in range(4)],
    3: [(j, None) for j in range(12)] + [(12 + d, d) for d in range(4)],
}

_CACHE = {}


def _build_nc():
    import concourse.tile as tile
    import concourse.mybir as mybir
    from concourse import bacc

    f32 = mybir.dt.float32
    f32r = mybir.dt.float32r

    nc = bacc.Bacc(None, target_bir_lowering=False)

    xT = nc.dram_tensor("xT", [C, T], f32r, kind="ExternalInput")
    wqk = nc.dram_tensor("wqk", [C, 2 * CL], f32r, kind="ExternalInput")
    wv = nc.dram_tensor("wv", [C, CL], f32r, kind="ExternalInput")
    wp = nc.dram_tensor("wp", [CL, C], f32r, kind="ExternalInput")
    cosP = nc.dram_tensor("cosP", [HD, T], f32, kind="ExternalInput")
    sinP = nc.dram_tensor("sinP", [HD, T], f32, kind="ExternalInput")
    rt = nc.dram_tensor("rt", [HD, HD], f32r, kind="ExternalInput")
    masks = nc.dram_tensor("masks", [4, 128, TC], f32r, kind="ExternalInput")
    ones = nc.dram_tensor("ones", [128, 1], f32r, kind="ExternalInput")
    out = nc.dram_tensor("out", [T, C], f32, kind="ExternalOutput")

    xT3 = xT.rearrange("(kt p) t -> p kt t", p=128)
    wqk3 = wqk.rearrange("(kt p) m -> p kt m", p=128)
    wv3 = wv.rearrange("(kt p) m -> p kt m", p=128)
    wp3 = wp.rearrange("(kt p) m -> p kt m", p=128)
    masks3 = masks.rearrange("d p n -> p d n")

    Exp = mybir.ActivationFunctionType.Exp

    with tile.TileContext(nc) as tc:
        # Pools are alloc'd/released manually (non-LIFO lifetimes) so DMA
        # prefetch for the next phase can be issued while the previous
        # phase's working set is still alive. Budget ~208 KB/partition.
        mpool = tc.alloc_tile_pool(name="misc", bufs=1)            # 9K whole
        qk_pool = tc.alloc_tile_pool(name="qkrope", bufs=1)        # 64K whole
        tpool = tc.alloc_tile_pool(name="trig", bufs=1, side="right")  # 16K A..rope

        rt_sb = mpool.tile([HD, HD], f32r)
        ones_sb = mpool.tile([128, 1], f32r)
        mask_sb = mpool.tile([128, 4, TC], f32r)
        cos_sb = tpool.tile([HD, T], f32)
        sin_sb = tpool.tile([HD, T], f32)

        # qkT[m] for m in 0..7: m<4 -> q head m, else k head m-4; [hd, T]
        # (rope outputs later reuse the same slots via identical tags)
        qkT = [qk_pool.tile([128, T], f32r, tag=f"qk{m}", name=f"qk{m}") for m in range(8)]

        # ---- stage A: qT/kT = W_{q,k}^T @ x^T, head-major tiles ----
        wpool = tc.alloc_tile_pool(name="wqk_sb", bufs=1)          # 64K A
        xpool = tc.alloc_tile_pool(name="xt_qk", bufs=1)           # 44K A
        ps1 = tc.alloc_tile_pool(name="ps_qk", bufs=4, space="PSUM")
        # interleave W / first-chunk x DMAs so PE can start accumulating
        # group (m=0, n=0) as soon as w[0]/x[0] land
        w_t = []
        x_first = []
        for k in range(KT):
            wt = wpool.tile([128, 2 * CL], f32r, tag=f"w{k}", name=f"w{k}")
            nc.sync.dma_start(out=wt, in_=wqk3[:, k])
            w_t.append(wt)
            xt = xpool.tile([128, TC], f32r, tag=f"x{k}", name=f"x{k}",
                            bufs=2 if k < 6 else 1)
            nc.sync.dma_start(out=xt, in_=xT3[:, k, 0:TC])
            x_first.append(xt)
        nc.sync.dma_start(out=rt_sb, in_=rt[:, :])
        nc.sync.dma_start(out=cos_sb, in_=cosP[:, :])
        nc.sync.dma_start(out=sin_sb, in_=sinP[:, :])
        for n in range(NT):
            if n == 0:
                x_t = x_first
            else:
                x_t = []
                for k in range(KT):
                    xt = xpool.tile([128, TC], f32r, tag=f"x{k}", name=f"x{k}",
                                    bufs=2 if k < 6 else 1)
                    nc.sync.dma_start(out=xt, in_=xT3[:, k, n * TC:(n + 1) * TC])
                    x_t.append(xt)
            for m in range(8):
                ps = ps1.tile([128, TC], f32, tag="ps_qk", name="ps_qk")
                for k in range(KT):
                    nc.tensor.matmul(
                        ps, w_t[k][:, m * 128:(m + 1) * 128], x_t[k],
                        start=(k == 0), stop=(k == KT - 1),
                    )
                nc.vector.tensor_copy(out=qkT[m][:, n * TC:(n + 1) * TC], in_=ps)
        xpool.release()
        wpool.release()
        ps1.release()

        # ---- stage B: RoPE on q and k (outputs reuse qk slots) ----
        # rope = qkT*cos + (R @ qkT)*sin ; R = pair swap w/ sign.
        # m-order (0,4,1,5,..) so head 0's q/k finish first and attention
        # can start while later heads still rope. v-phase DMAs (wv, xv)
        # are issued up front so v matmuls overlap RoPE's DVE work.
        v_pool = tc.alloc_tile_pool(name="v_sb", bufs=1)           # 32K ..attn
        xvpool = tc.alloc_tile_pool(name="xt_v", bufs=2)           # 32K ..v
        wvpool = tc.alloc_tile_pool(name="wv_sb", bufs=1)          # 32K ..v
        v_t = [v_pool.tile([128, CL], f32r, tag=f"v{mt}", name=f"v{mt}")
               for mt in range(TT)]
        wv_t = []
        for k in range(KT):
            wt = wvpool.tile([128, CL], f32r, tag=f"wv{k}", name=f"wv{k}")
            nc.sync.dma_start(out=wt, in_=wv3[:, k])
            wv_t.append(wt)
        xv_t = {}
        for pair in range(TT // 2):
            for k in range(KT):
                xt = xvpool.tile([128, 256], f32r, tag=f"xv{k}", name=f"xv{k}")
                nc.sync.dma_start(
                    out=xt, in_=xT3[:, k, pair * 256:(pair + 1) * 256]
                )
                xv_t[(pair, k)] = xt

        rope = [None] * 8
        rtmp = tc.alloc_tile_pool(name="rope_tmp", bufs=4)
        psr = tc.alloc_tile_pool(name="ps_rot", bufs=4, space="PSUM")
        for m in (0, 4, 1, 5, 2, 6, 3, 7):
            tmp = []
            for n in range(NT):
                sl = slice(n * TC, (n + 1) * TC)
                ps = psr.tile([128, TC], f32, tag="ps_rot", name="ps_rot")
                nc.tensor.matmul(ps, rt_sb, qkT[m][:, sl], start=True, stop=True)
                t1 = rtmp.tile([128, TC], f32, tag="t1", name="t1")
                t2 = rtmp.tile([128, TC], f32, tag="t2", name="t2")
                nc.vector.tensor_mul(t1, ps, sin_sb[:, sl])
                nc.vector.tensor_mul(t2, qkT[m][:, sl], cos_sb[:, sl])
                tmp.append((t1, t2))
            # all reads of qkT[m] issued; now write into its slot
            ro = qk_pool.tile([128, T], f32r, tag=f"qk{m}", name=f"rope{m}")
            for n in range(NT):
                sl = slice(n * TC, (n + 1) * TC)
                nc.vector.tensor_add(ro[:, sl], tmp[n][0], tmp[n][1])
            rope[m] = ro
        rtmp.release()
        psr.release()
        tpool.release()

        # ---- stage C: v = x @ Wv (natural layout), all 4 heads ----
        ps2 = tc.alloc_tile_pool(name="ps_v", bufs=4, space="PSUM")
        nc.sync.dma_start(out=ones_sb, in_=ones[:, :])
        nc.sync.dma_start(out=mask_sb, in_=masks3)
        for pair in range(TT // 2):
            for half in range(2):
                mt = 2 * pair + half
                ps = ps2.tile([128, CL], f32, tag="ps_v", name="ps_v")
                for k in range(KT):
                    nc.tensor.matmul(
                        ps, xv_t[(pair, k)][:, half * 128:(half + 1) * 128],
                        wv_t[k], start=(k == 0), stop=(k == KT - 1),
                    )
                nc.vector.tensor_copy(out=v_t[mt], in_=ps)
        wvpool.release()
        xvpool.release()
        ps2.release()

        # ---- stage D: attention; stage E (proj) overlaps its tail ----
        y_pool = tc.alloc_tile_pool(name="yT_sb", bufs=1)          # 32K
        yT = [y_pool.tile([128, T], f32r, tag=f"yT{h}", name=f"yT{h}")
              for h in range(HPC)]
        wppool = tc.alloc_tile_pool(name="wp_sb", bufs=1)          # 32K
        wp_t = []
        for hk in range(HPC):
            wt = wppool.tile([128, C], f32r, tag=f"wp{hk}", name=f"wp{hk}")
            nc.sync.dma_start(out=wt, in_=wp3[:, hk])
            wp_t.append(wt)

        pp_pool = tc.alloc_tile_pool(name="pp", bufs=5)
        sm_pool = tc.alloc_tile_pool(name="small", bufs=2)
        ps_s = tc.alloc_tile_pool(name="ps_s", bufs=2, space="PSUM")
        ps_y = tc.alloc_tile_pool(name="ps_y", bufs=2, space="PSUM")
        ps_d = tc.alloc_tile_pool(name="ps_d", bufs=2, space="PSUM")
        ps_o = tc.alloc_tile_pool(name="ps_o", bufs=2, space="PSUM")
        opool = tc.alloc_tile_pool(name="ostage", bufs=3)

        for h in range(HPC):
            q_h = rope[h]
            k_h = rope[4 + h]
            for I in range(NT):
                isl = slice(I * TC, (I + 1) * TC)
                jl = _JLISTS[I]
                y_ps = ps_y.tile([128, TC], f32, tag="y", name="y_ps")
                d_ps = ps_d.tile([1, TC], f32, tag="d", name="d_ps")
                for jidx, (J, d) in enumerate(jl):
                    s_ps = ps_s.tile([128, TC], f32, tag="s", name="s_ps")
                    nc.tensor.matmul(
                        s_ps, k_h[:, J * 128:(J + 1) * 128],
                        q_h[:, isl], start=True, stop=True,
                    )
                    pp = pp_pool.tile([128, TC], f32r, tag="pp", name="pp")
                    nc.scalar.activation(out=pp, in_=s_ps, func=Exp, scale=SCALE)
                    if d is not None:
                        ppm = pp_pool.tile([128, TC], f32r, tag="ppm",
                                           name="ppm", bufs=2)
                        nc.vector.tensor_mul(ppm, pp, mask_sb[:, d])
                        pp = ppm
                    first = jidx == 0
                    last = jidx == len(jl) - 1
                    nc.tensor.matmul(
                        y_ps, v_t[J][:, h * 128:(h + 1) * 128], pp,
                        start=first, stop=last,
                    )
                    nc.tensor.matmul(d_ps, ones_sb, pp, start=first, stop=last)
                recip = sm_pool.tile([1, TC], f32, tag="recip", name="recip")
                nc.vector.reciprocal(out=recip, in_=d_ps)
                recipB = sm_pool.tile([128, TC], f32, tag="recipB", name="recipB")
                nc.gpsimd.partition_broadcast(recipB, recip)
                nc.vector.tensor_mul(yT[h][:, isl], y_ps, recipB)

        # ---- stage E: partial out = yT^T @ Wp, grouped by query chunk so
        # chunks whose yT rows are complete overlap the remaining attention
        for I in range(NT):
            for ml in range(4):
                mt = 4 * I + ml
                msl = slice(mt * 128, (mt + 1) * 128)
                for n in range(NT):
                    ps = ps_o.tile([128, TC], f32, tag="o", name="o_ps")
                    for hk in range(HPC):
                        nc.tensor.matmul(
                            ps, yT[hk][:, msl], wp_t[hk][:, n * TC:(n + 1) * TC],
                            start=(hk == 0), stop=(hk == HPC - 1),
                        )
                    ot = opool.tile([128, TC], f32, tag="ot", name="ot")
                    nc.scalar.copy(out=ot, in_=ps)
                    nc.sync.dma_start(out=out[msl, n * TC:(n + 1) * TC], in_=ot)

        for p in (opool, sm_pool, pp_pool, wppool, y_pool, v_pool,
                  qk_pool, mpool, ps_o, ps_d, ps_y, ps_s):
            p.release()
    nc.compile()
    return nc


def _host_prep(x, w_qkv, w_proj, freqs_cis):
    """Build per-core input maps (slicing + layout prep only)."""
    x = np.asarray(x, dtype=np.float32)
    w_qkv = np.asarray(w_qkv, dtype=np.float32)
    w_proj = np.asarray(w_proj, dtype=np.float32)
    fc = np.asarray(freqs_cis, dtype=np.float32)

    xTb = [np.ascontiguousarray(x[b].T) for b in range(B)]

    cos = fc[:, :, 0].T  # [64, T]
    sin = fc[:, :, 1].T
    cosP = np.repeat(cos, 2, axis=0).astype(np.float32)  # [128, T]
    sinP = np.repeat(sin, 2, axis=0).astype(np.float32)

    rt = np.zeros((HD, HD), dtype=np.float32)
    for d in range(HD // 2):
        rt[2 * d, 2 * d + 1] = 1.0
        rt[2 * d + 1, 2 * d] = -1.0

    masks = np.zeros((4, 128, TC), dtype=np.float32)
    ii = np.arange(TC)[None, :]
    jj = np.arange(128)[:, None]
    for d in range(4):
        masks[d] = (ii >= jj + 128 * d).astype(np.float32)

    ones = np.ones((128, 1), dtype=np.float32)

    in_maps = []
    for core in range(8):
        b = core // 4
        g = core % 4
        qc = np.ascontiguousarray(w_qkv[:, 512 * g: 512 * (g + 1)])
        kc = np.ascontiguousarray(w_qkv[:, 2048 + 512 * g: 2048 + 512 * (g + 1)])
        vc = np.ascontiguousarray(w_qkv[:, 4096 + 512 * g: 4096 + 512 * (g + 1)])
        wqk_c = np.concatenate([qc, kc], axis=1)
        wp_c = np.ascontiguousarray(w_proj[512 * g: 512 * (g + 1), :])
        in_maps.append({
            "xT": xTb[b],
            "wqk": wqk_c,
            "wv": vc,
            "wp": wp_c,
            "cosP": cosP,
            "sinP": sinP,
            "rt": rt,
            "masks": masks,
            "ones": ones,
        })
    return in_maps


def _get_nc():
    if "nc" not in _CACHE:
        _CACHE["nc"] = _build_nc()
    return _CACHE["nc"]


def kernel(x, w_qkv, w_proj, freqs_cis, attn_mask, _trace=False):
    from concourse.bass_utils import run_bass_kernel_spmd

    in_maps = _host_prep(x, w_qkv, w_proj, freqs_cis)
    nc = _get_nc()
    res = run_bass_kernel_spmd(
        nc, in_maps, core_ids=list(range(8)), trace=_trace,
    )
    outs = [r["out"].astype(np.float64) for r in res.results]
    full = np.stack([
        outs[0] + outs[1] + outs[2] + outs[3],
        outs[4] + outs[5] + outs[6] + outs[7],
    ]).astype(np.float32)
    if _trace:
        kernel._last_results = res
    return full



# revision 10
# speedup vs baseline: 1.3702x; 1.3702x over previous
"""Trainium2 Bass kernel for prefix-LM CausalSelfAttention.

Problem: B=2, T=2048, C=2048, H=16 heads (hd=128), prefix-LM mask
(bidirectional over first half, causal after), RoPE on q/k.

Sharding over 8 cores: data-parallel on batch (2) x tensor-parallel on
heads (4 heads per core). Each core computes a partial output projection
(its heads' contribution); partials are summed on host.

All matmul operands are bf16 (PE full rate, f32 PSUM accumulation);
only the final output DMA is f32. Per-core dataflow:
  1. qT/kT = W^T @ x^T    [hd*4, T] head-major tiles; x loaded once
     (bf16) and kept resident for both this and the v matmuls.
  2. RoPE (pair-swap matmul + DVE combine) interleaved with
     v = x @ Wv so the PE never idles while the DVE ropes.
  3. Attention per (chunk I, head h), software-pipelined one key-tile
     ahead so exp (ACT) hides under the PE matmuls:
       S'[J] = k[:,J]^T x q[:,I]   (scores transposed, [j,i])
       P'[J] = exp(S' * scale)     (ACT, psum->sbuf bf16)
       mask-multiply for diagonal-crossing tiles (4 static patterns)
       y_psum += v[J,h]^T x P'[J]
       d_psum += ones128^T x P'[J]  -> full [128,512] broadcast rowsum
     yT[:, I] = y_psum * reciprocal_approx_fast(d_psum)
  4. After each chunk I's 4 heads: partial out rows = yT^T @ Wp.

Fully-masked key tiles are skipped (structural sparsity: 44/64 tiles).
"""
import math

import numpy as np

N_HEAD = 16
B = 2
T = 2048
C = 2048
HD = 128
HPC = 4          # heads per core
CL = HPC * HD    # local C = 512
TC = 512         # chunk width (matmul moving free dim / psum bank)
NT = T // TC     # 4 chunks
KT = C // 128    # 16 contraction tiles over C
TT = T // 128    # 16 T tiles
SCALE = 1.0 / math.sqrt(HD)

# Per query-chunk I: list of (J, mask_idx) key tiles to compute.
# mask_idx is None for fully-allowed tiles, else 0..3 selecting the
# static diagonal pattern mask[d][jj, ii] = (ii >= jj + 128*d).
_JLISTS = {
    0: [(j, None) for j in range(8)],
    1: [(j, None) for j in range(8)],
    2: [(j, None) for j in range(8)] + [(8 + d, d) for d in range(4)],
    3: [(j, None) for j in range(12)] + [(12 + d, d) for d in range(4)],
}

_CACHE = {}


def _build_nc():
    import concourse.tile as tile
    import concourse.mybir as mybir
    from concourse import bacc

    f32 = mybir.dt.float32
    bf16 = mybir.dt.bfloat16

    nc = bacc.Bacc(None, target_bir_lowering=False)

    xT = nc.dram_tensor("xT", [C, T], bf16, kind="ExternalInput")
    wqk = nc.dram_tensor("wqk", [C, 2 * CL], bf16, kind="ExternalInput")
    wv = nc.dram_tensor("wv", [C, CL], bf16, kind="ExternalInput")
    wp = nc.dram_tensor("wp", [CL, C], bf16, kind="ExternalInput")
    cosP = nc.dram_tensor("cosP", [HD, T], bf16, kind="ExternalInput")
    sinP = nc.dram_tensor("sinP", [HD, T], bf16, kind="ExternalInput")
    rt = nc.dram_tensor("rt", [HD, HD], bf16, kind="ExternalInput")
    masks = nc.dram_tensor("masks", [4, 128, TC], bf16, kind="ExternalInput")
    ones = nc.dram_tensor("ones", [128, 128], bf16, kind="ExternalInput")
    out = nc.dram_tensor("out", [T, C], f32, kind="ExternalOutput")

    xT3 = xT.rearrange("(kt p) t -> p kt t", p=128)
    wqk3 = wqk.rearrange("(kt p) m -> p kt m", p=128)
    wv3 = wv.rearrange("(kt p) m -> p kt m", p=128)
    wp3 = wp.rearrange("(kt p) m -> p kt m", p=128)
    masks3 = masks.rearrange("d p n -> p d n")

    Exp = mybir.ActivationFunctionType.Exp

    with tile.TileContext(nc) as tc:
        mpool = tc.alloc_tile_pool(name="misc", bufs=1)
        qk_pool = tc.alloc_tile_pool(name="qkrope", bufs=1)
        tpool = tc.alloc_tile_pool(name="trig", bufs=1, side="right")

        rt_sb = mpool.tile([HD, HD], bf16)
        ones_sb = mpool.tile([128, 128], bf16)
        mask_sb = mpool.tile([128, 4, TC], bf16)
        cos_sb = tpool.tile([HD, T], bf16)
        sin_sb = tpool.tile([HD, T], bf16)

        # qkT[m] for m in 0..7: m<4 -> q head m, else k head m-4; [hd, T]
        # (rope outputs later reuse the same slots via identical tags)
        qkT = [qk_pool.tile([128, T], bf16, tag=f"qk{m}", name=f"qk{m}") for m in range(8)]

        # Long-lived pools first (pool release must be LIFO per side):
        # yT / Wp / v live to the end; x / wv / wqk release after stage C.
        y_pool = tc.alloc_tile_pool(name="yT_sb", bufs=1)          # 16K
        yT = [y_pool.tile([128, T], bf16, tag=f"yT{h}", name=f"yT{h}")
              for h in range(HPC)]
        wppool = tc.alloc_tile_pool(name="wp_sb", bufs=1)          # 16K
        v_pool = tc.alloc_tile_pool(name="v_sb", bufs=1)           # 16K ..attn
        v_t = [v_pool.tile([128, CL], bf16, tag=f"v{mt}", name=f"v{mt}")
               for mt in range(TT)]

        # ---- stage A: qT/kT = W_{q,k}^T @ x^T, head-major tiles ----
        # x (bf16) is loaded ONCE, fully resident; stage C reuses it.
        xpool = tc.alloc_tile_pool(name="xt_all", bufs=1)          # 64K A..C
        wvpool = tc.alloc_tile_pool(name="wv_sb", bufs=1)          # 16K ..v
        wpool = tc.alloc_tile_pool(name="wqk_sb", bufs=1)          # 32K A
        ps1 = tc.alloc_tile_pool(name="ps_qk", bufs=4, space="PSUM")
        w_t = []
        x_t = {}
        for k in range(KT):
            wt = wpool.tile([128, 2 * CL], bf16, tag=f"w{k}", name=f"w{k}")
            nc.sync.dma_start(out=wt, in_=wqk3[:, k])
            w_t.append(wt)
            xt = xpool.tile([128, TC], bf16, tag=f"x0_{k}", name=f"x0_{k}")
            nc.sync.dma_start(out=xt, in_=xT3[:, k, 0:TC])
            x_t[(0, k)] = xt
        nc.sync.dma_start(out=rt_sb, in_=rt[:, :])
        nc.sync.dma_start(out=cos_sb, in_=cosP[:, :])
        nc.sync.dma_start(out=sin_sb, in_=sinP[:, :])
        for n in range(1, NT):
            for k in range(KT):
                xt = xpool.tile([128, TC], bf16, tag=f"x{n}_{k}", name=f"x{n}_{k}")
                nc.sync.dma_start(out=xt, in_=xT3[:, k, n * TC:(n + 1) * TC])
                x_t[(n, k)] = xt

        # v-phase + attention constants DMA'd early (all overlap stage A)
        wv_t = []
        for k in range(KT):
            wt = wvpool.tile([128, CL], bf16, tag=f"wv{k}", name=f"wv{k}")
            nc.sync.dma_start(out=wt, in_=wv3[:, k])
            wv_t.append(wt)
        nc.sync.dma_start(out=ones_sb, in_=ones[:, :])
        nc.sync.dma_start(out=mask_sb, in_=masks3)
        wp_t = []
        for hk in range(HPC):
            wt = wppool.tile([128, C], bf16, tag=f"wp{hk}", name=f"wp{hk}")
            nc.sync.dma_start(out=wt, in_=wp3[:, hk])
            wp_t.append(wt)

        for n in range(NT):
            for m in range(8):
                ps = ps1.tile([128, TC], f32, tag="ps_qk", name="ps_qk")
                for k in range(KT):
                    nc.tensor.matmul(
                        ps, w_t[k][:, m * 128:(m + 1) * 128], x_t[(n, k)],
                        start=(k == 0), stop=(k == KT - 1),
                    )
                nc.vector.tensor_copy(out=qkT[m][:, n * TC:(n + 1) * TC], in_=ps)
        wpool.release()
        ps1.release()

        # ---- stage B+C interleaved: RoPE (DVE-heavy) + v = x @ Wv
        # (PE-heavy) so the PE never waits on the DVE rope chain.
        # rope = qkT*cos + (R @ qkT)*sin ; R = pair swap w/ sign.
        rtmp = tc.alloc_tile_pool(name="rope_tmp", bufs=4)
        psr = tc.alloc_tile_pool(name="ps_rot", bufs=4, space="PSUM")
        ps2 = tc.alloc_tile_pool(name="ps_v", bufs=4, space="PSUM")
        rope = [None] * 8
        for m in range(8):
            tmp = []
            for n in range(NT):
                sl = slice(n * TC, (n + 1) * TC)
                ps = psr.tile([128, TC], f32, tag="ps_rot", name="ps_rot")
                nc.tensor.matmul(ps, rt_sb, qkT[m][:, sl], start=True, stop=True)
                t1 = rtmp.tile([128, TC], bf16, tag="t1", name="t1")
                t2 = rtmp.tile([128, TC], bf16, tag="t2", name="t2")
                nc.vector.tensor_mul(t1, ps, sin_sb[:, sl])
                nc.vector.tensor_mul(t2, qkT[m][:, sl], cos_sb[:, sl])
                tmp.append((t1, t2))
            # all reads of qkT[m] issued; now write into its slot
            ro = qk_pool.tile([128, T], bf16, tag=f"qk{m}", name=f"rope{m}")
            for n in range(NT):
                sl = slice(n * TC, (n + 1) * TC)
                nc.vector.tensor_add(ro[:, sl], tmp[n][0], tmp[n][1])
            rope[m] = ro
            # two v T-tiles per rope head: PE work covering the DVE chain
            for mt in (2 * m, 2 * m + 1):
                nv = mt // 4
                off = (mt % 4) * 128
                ps = ps2.tile([128, CL], f32, tag="ps_v", name="ps_v")
                for k in range(KT):
                    nc.tensor.matmul(
                        ps, x_t[(nv, k)][:, off:off + 128],
                        wv_t[k], start=(k == 0), stop=(k == KT - 1),
                    )
                nc.scalar.copy(out=v_t[mt], in_=ps)
        rtmp.release()
        wvpool.release()
        xpool.release()
        tpool.release()
        ps2.release()
        psr.release()

        # ---- stage D: attention (I outer, h inner), one-tile software
        # pipeline: the PE stream is S0,S1,AV0,d0,S2,AV1,d1,... so exp[j]
        # (ACT) runs while the PE does S[j+1]. Stage E (proj) for chunk I
        # follows its 4 heads, keeping ACT/DVE load smooth.
        pp_pool = tc.alloc_tile_pool(name="pp", bufs=5)
        sm_pool = tc.alloc_tile_pool(name="small", bufs=2)
        ps_s = tc.alloc_tile_pool(name="ps_s", bufs=2, space="PSUM")
        ps_y = tc.alloc_tile_pool(name="ps_y", bufs=2, space="PSUM")
        ps_d = tc.alloc_tile_pool(name="ps_d", bufs=2, space="PSUM")
        ps_o = tc.alloc_tile_pool(name="ps_o", bufs=2, space="PSUM")
        opool = tc.alloc_tile_pool(name="ostage", bufs=3)

        for I in range(NT):
            isl = slice(I * TC, (I + 1) * TC)
            jl = _JLISTS[I]
            nj = len(jl)
            for h in range(HPC):
                q_h = rope[h]
                k_h = rope[4 + h]
                y_ps = ps_y.tile([128, TC], f32, tag="y", name="y_ps")
                d_ps = ps_d.tile([128, TC], f32, tag="d", name="d_ps")

                def emit_S(jidx):
                    J = jl[jidx][0]
                    s_ps = ps_s.tile([128, TC], f32, tag="s", name="s_ps")
                    nc.tensor.matmul(
                        s_ps, k_h[:, J * 128:(J + 1) * 128],
                        q_h[:, isl], start=True, stop=True,
                    )
                    pp = pp_pool.tile([128, TC], bf16, tag="pp", name="pp")
                    nc.scalar.activation(out=pp, in_=s_ps, func=Exp, scale=SCALE)
                    return pp

                pps = [None] * nj
                pps[0] = emit_S(0)
                for jidx, (J, dm) in enumerate(jl):
                    if jidx + 1 < nj:
                        pps[jidx + 1] = emit_S(jidx + 1)
                    pp = pps[jidx]
                    if dm is not None:
                        ppm = pp_pool.tile([128, TC], bf16, tag="ppm",
                                           name="ppm", bufs=2)
                        nc.vector.tensor_mul(ppm, pp, mask_sb[:, dm])
                        pp = ppm
                    first = jidx == 0
                    last = jidx == nj - 1
                    nc.tensor.matmul(
                        y_ps, v_t[J][:, h * 128:(h + 1) * 128], pp,
                        start=first, stop=last,
                    )
                    nc.tensor.matmul(d_ps, ones_sb, pp, start=first, stop=last)
                recip = sm_pool.tile([128, TC], f32, tag="recip", name="recip")
                nc.vector.reciprocal_approx_fast(out=recip, in_=d_ps)
                nc.vector.tensor_mul(yT[h][:, isl], y_ps, recip)

            # ---- stage E for chunk I: out rows = yT^T @ Wp ----
            for ml in range(4):
                mt = 4 * I + ml
                msl = slice(mt * 128, (mt + 1) * 128)
                for n in range(NT):
                    ps = ps_o.tile([128, TC], f32, tag="o", name="o_ps")
                    for hk in range(HPC):
                        nc.tensor.matmul(
                            ps, yT[hk][:, msl], wp_t[hk][:, n * TC:(n + 1) * TC],
                            start=(hk == 0), stop=(hk == HPC - 1),
                        )
                    ot = opool.tile([128, TC], f32, tag="ot", name="ot")
                    nc.scalar.copy(out=ot, in_=ps)
                    nc.sync.dma_start(out=out[msl, n * TC:(n + 1) * TC], in_=ot)

        for p in (opool, sm_pool, pp_pool, v_pool, wppool, y_pool,
                  qk_pool, mpool, ps_o, ps_d, ps_y, ps_s):
            p.release()
    nc.compile()
    return nc


def _host_prep(x, w_qkv, w_proj, freqs_cis):
    """Build per-core input maps (slicing + layout prep only)."""
    import ml_dtypes
    bf16 = ml_dtypes.bfloat16

    x = np.asarray(x, dtype=np.float32)
    w_qkv = np.asarray(w_qkv, dtype=np.float32)
    w_proj = np.asarray(w_proj, dtype=np.float32)
    fc = np.asarray(freqs_cis, dtype=np.float32)

    xTb = [np.ascontiguousarray(x[b].T).astype(bf16) for b in range(B)]

    cos = fc[:, :, 0].T  # [64, T]
    sin = fc[:, :, 1].T
    cosP = np.repeat(cos, 2, axis=0).astype(bf16)  # [128, T]
    sinP = np.repeat(sin, 2, axis=0).astype(bf16)

    rt = np.zeros((HD, HD), dtype=np.float32)
    for d in range(HD // 2):
        rt[2 * d, 2 * d + 1] = 1.0
        rt[2 * d + 1, 2 * d] = -1.0
    rt = rt.astype(bf16)

    masks = np.zeros((4, 128, TC), dtype=np.float32)
    ii = np.arange(TC)[None, :]
    jj = np.arange(128)[:, None]
    for d in range(4):
        masks[d] = (ii >= jj + 128 * d).astype(np.float32)
    masks = masks.astype(bf16)

    ones = np.ones((128, 128), dtype=bf16)

    in_maps = []
    for core in range(8):
        b = core // 4
        g = core % 4
        qc = w_qkv[:, 512 * g: 512 * (g + 1)]
        kc = w_qkv[:, 2048 + 512 * g: 2048 + 512 * (g + 1)]
        vc = np.ascontiguousarray(w_qkv[:, 4096 + 512 * g: 4096 + 512 * (g + 1)]).astype(bf16)
        wqk_c = np.concatenate([qc, kc], axis=1).astype(bf16)
        wp_c = np.ascontiguousarray(w_proj[512 * g: 512 * (g + 1), :]).astype(bf16)
        in_maps.append({
            "xT": xTb[b],
            "wqk": wqk_c,
            "wv": vc,
            "wp": wp_c,
            "cosP": cosP,
            "sinP": sinP,
            "rt": rt,
            "masks": masks,
            "ones": ones,
        })
    return in_maps


def _get_nc():
    if "nc" not in _CACHE:
        _CACHE["nc"] = _build_nc()
    return _CACHE["nc"]


def kernel(x, w_qkv, w_proj, freqs_cis, attn_mask, _trace=False):
    from concourse.bass_utils import run_bass_kernel_spmd

    in_maps = _host_prep(x, w_qkv, w_proj, freqs_cis)
    nc = _get_nc()
    res = run_bass_kernel_spmd(
        nc, in_maps, core_ids=list(range(8)), trace=_trace,
    )
    outs = [r["out"].astype(np.float64) for r in res.results]
    full = np.stack([
        outs[0] + outs[1] + outs[2] + outs[3],
        outs[4] + outs[5] + outs[6] + outs[7],
    ]).astype(np.float32)
    if _trace:
        kernel._last_results = res
    return full


# revision 11
# speedup vs baseline: 1.3978x; 1.0202x over previous
"""Trainium2 Bass kernel for prefix-LM CausalSelfAttention.

Problem: B=2, T=2048, C=2048, H=16 heads (hd=128), prefix-LM mask
(bidirectional over first half, causal after), RoPE on q/k.

Sharding over 8 cores: data-parallel on batch (2) x tensor-parallel on
heads (4 heads per core). Each core computes a partial output projection
(its heads' contribution); partials are summed on host.

All matmul operands are bf16 (PE full rate, f32 PSUM accumulation);
only the final output DMA is f32. Per-core dataflow:
  1. qT/kT = W^T @ x^T    [hd*4, T] head-major tiles; x loaded once
     (bf16) and kept resident for both this and the v matmuls.
  2. RoPE (pair-swap matmul + DVE combine) interleaved with
     v = x @ Wv so the PE never idles while the DVE ropes.
  3. Attention per (chunk I, head h), software-pipelined one key-tile
     ahead so exp (ACT) hides under the PE matmuls:
       S'[J] = k[:,J]^T x q[:,I]   (scores transposed, [j,i])
       P'[J] = exp(S' * scale)     (ACT, psum->sbuf bf16)
       mask-multiply for diagonal-crossing tiles (4 static patterns)
       y_psum += v[J,h]^T x P'[J]
       d_psum += ones128^T x P'[J]  -> full [128,512] broadcast rowsum
     yT[:, I] = y_psum * reciprocal_approx_fast(d_psum)
  4. After each chunk I's 4 heads: partial out rows = yT^T @ Wp.

Fully-masked key tiles are skipped (structural sparsity: 44/64 tiles).
"""
import math

import numpy as np

N_HEAD = 16
B = 2
T = 2048
C = 2048
HD = 128
HPC = 4          # heads per core
CL = HPC * HD    # local C = 512
TC = 512         # chunk width (matmul moving free dim / psum bank)
NT = T // TC     # 4 chunks
KT = C // 128    # 16 contraction tiles over C
TT = T // 128    # 16 T tiles
SCALE = 1.0 / math.sqrt(HD)

# Per query-chunk I: list of (J, mask_idx) key tiles to compute.
# mask_idx is None for fully-allowed tiles, else 0..3 selecting the
# static diagonal pattern mask[d][jj, ii] = (ii >= jj + 128*d).
_JLISTS = {
    0: [(j, None) for j in range(8)],
    1: [(j, None) for j in range(8)],
    2: [(j, None) for j in range(8)] + [(8 + d, d) for d in range(4)],
    3: [(j, None) for j in range(12)] + [(12 + d, d) for d in range(4)],
}

_CACHE = {}


def _build_nc():
    import concourse.tile as tile
    import concourse.mybir as mybir
    from concourse import bacc

    f32 = mybir.dt.float32
    bf16 = mybir.dt.bfloat16

    nc = bacc.Bacc(None, target_bir_lowering=False)

    xT = nc.dram_tensor("xT", [C, T], bf16, kind="ExternalInput")
    wqk = nc.dram_tensor("wqk", [C, 2 * CL], bf16, kind="ExternalInput")
    wv = nc.dram_tensor("wv", [C, CL], bf16, kind="ExternalInput")
    wp = nc.dram_tensor("wp", [CL, C], bf16, kind="ExternalInput")
    cosP = nc.dram_tensor("cosP", [HD, T], bf16, kind="ExternalInput")
    sinP = nc.dram_tensor("sinP", [HD, T], bf16, kind="ExternalInput")
    rt = nc.dram_tensor("rt", [HD, HD], bf16, kind="ExternalInput")
    masks = nc.dram_tensor("masks", [4, 128, TC], bf16, kind="ExternalInput")
    ones = nc.dram_tensor("ones", [128, 128], bf16, kind="ExternalInput")
    out = nc.dram_tensor("out", [T, C], f32, kind="ExternalOutput")

    xT3 = xT.rearrange("(kt p) t -> p kt t", p=128)
    wqk3 = wqk.rearrange("(kt p) m -> p kt m", p=128)
    wv3 = wv.rearrange("(kt p) m -> p kt m", p=128)
    wp3 = wp.rearrange("(kt p) m -> p kt m", p=128)
    masks3 = masks.rearrange("d p n -> p d n")

    Exp = mybir.ActivationFunctionType.Exp

    with tile.TileContext(nc) as tc:
        mpool = tc.alloc_tile_pool(name="misc", bufs=1)
        qk_pool = tc.alloc_tile_pool(name="qkrope", bufs=1)
        tpool = tc.alloc_tile_pool(name="trig", bufs=1, side="right")

        rt_sb = mpool.tile([HD, HD], bf16)
        ones_sb = mpool.tile([128, 128], bf16)
        mask_sb = mpool.tile([128, 4, TC], bf16)
        cos_sb = tpool.tile([HD, T], bf16)
        sin_sb = tpool.tile([HD, T], bf16)

        # qkT[m] for m in 0..7: m<4 -> q head m, else k head m-4; [hd, T]
        # (rope outputs later reuse the same slots via identical tags)
        qkT = [qk_pool.tile([128, T], bf16, tag=f"qk{m}", name=f"qk{m}") for m in range(8)]

        # Long-lived pools first (pool release must be LIFO per side):
        # yT / Wp / v live to the end; x / wv / wqk release after stage C.
        y_pool = tc.alloc_tile_pool(name="yT_sb", bufs=1)          # 16K
        yT = [y_pool.tile([128, T], bf16, tag=f"yT{h}", name=f"yT{h}")
              for h in range(HPC)]
        wppool = tc.alloc_tile_pool(name="wp_sb", bufs=1)          # 16K
        v_pool = tc.alloc_tile_pool(name="v_sb", bufs=1)           # 16K ..attn
        v_t = [v_pool.tile([128, CL], bf16, tag=f"v{mt}", name=f"v{mt}")
               for mt in range(TT)]

        # ---- stage A: qT/kT = W_{q,k}^T @ x^T, head-major tiles ----
        # x (bf16) is loaded ONCE, fully resident; stage C reuses it.
        xpool = tc.alloc_tile_pool(name="xt_all", bufs=1)          # 64K A..C
        wvpool = tc.alloc_tile_pool(name="wv_sb", bufs=1)          # 16K ..v
        wpool = tc.alloc_tile_pool(name="wqk_sb", bufs=1)          # 32K A
        ps1 = tc.alloc_tile_pool(name="ps_qk", bufs=4, space="PSUM")
        w_t = []
        x_t = {}
        for k in range(KT):
            wt = wpool.tile([128, 2 * CL], bf16, tag=f"w{k}", name=f"w{k}")
            nc.sync.dma_start(out=wt, in_=wqk3[:, k])
            w_t.append(wt)
            xt = xpool.tile([128, TC], bf16, tag=f"x0_{k}", name=f"x0_{k}")
            nc.sync.dma_start(out=xt, in_=xT3[:, k, 0:TC])
            x_t[(0, k)] = xt
        nc.sync.dma_start(out=rt_sb, in_=rt[:, :])
        nc.sync.dma_start(out=cos_sb, in_=cosP[:, :])
        nc.sync.dma_start(out=sin_sb, in_=sinP[:, :])
        for n in range(1, NT):
            for k in range(KT):
                xt = xpool.tile([128, TC], bf16, tag=f"x{n}_{k}", name=f"x{n}_{k}")
                nc.sync.dma_start(out=xt, in_=xT3[:, k, n * TC:(n + 1) * TC])
                x_t[(n, k)] = xt

        # v-phase + attention constants DMA'd early (all overlap stage A)
        wv_t = []
        for k in range(KT):
            wt = wvpool.tile([128, CL], bf16, tag=f"wv{k}", name=f"wv{k}")
            nc.sync.dma_start(out=wt, in_=wv3[:, k])
            wv_t.append(wt)
        nc.sync.dma_start(out=ones_sb, in_=ones[:, :])
        nc.sync.dma_start(out=mask_sb, in_=masks3)
        wp_t = []
        for hk in range(HPC):
            wt = wppool.tile([128, C], bf16, tag=f"wp{hk}", name=f"wp{hk}")
            nc.sync.dma_start(out=wt, in_=wp3[:, hk])
            wp_t.append(wt)

        for n in range(NT):
            for m in range(8):
                ps = ps1.tile([128, TC], f32, tag="ps_qk", name="ps_qk")
                for k in range(KT):
                    nc.tensor.matmul(
                        ps, w_t[k][:, m * 128:(m + 1) * 128], x_t[(n, k)],
                        start=(k == 0), stop=(k == KT - 1),
                    )
                nc.vector.tensor_copy(out=qkT[m][:, n * TC:(n + 1) * TC], in_=ps)
        wpool.release()
        ps1.release()

        # ---- stage B+C interleaved: RoPE (DVE-heavy) + v = x @ Wv
        # (PE-heavy) so the PE never waits on the DVE rope chain.
        # rope = qkT*cos + (R @ qkT)*sin ; R = pair swap w/ sign.
        rtmp = tc.alloc_tile_pool(name="rope_tmp", bufs=4)
        psr = tc.alloc_tile_pool(name="ps_rot", bufs=4, space="PSUM")
        ps2 = tc.alloc_tile_pool(name="ps_v", bufs=4, space="PSUM")
        rope = [None] * 8
        for m in range(8):
            tmp = []
            for n in range(NT):
                sl = slice(n * TC, (n + 1) * TC)
                ps = psr.tile([128, TC], f32, tag="ps_rot", name="ps_rot")
                nc.tensor.matmul(ps, rt_sb, qkT[m][:, sl], start=True, stop=True)
                t1 = rtmp.tile([128, TC], bf16, tag="t1", name="t1")
                t2 = rtmp.tile([128, TC], bf16, tag="t2", name="t2")
                nc.vector.tensor_mul(t1, ps, sin_sb[:, sl])
                nc.vector.tensor_mul(t2, qkT[m][:, sl], cos_sb[:, sl])
                tmp.append((t1, t2))
            # all reads of qkT[m] issued; now write into its slot
            ro = qk_pool.tile([128, T], bf16, tag=f"qk{m}", name=f"rope{m}")
            for n in range(NT):
                sl = slice(n * TC, (n + 1) * TC)
                nc.vector.tensor_add(ro[:, sl], tmp[n][0], tmp[n][1])
            rope[m] = ro
            # two v T-tiles per rope head: PE work covering the DVE chain
            for mt in (2 * m, 2 * m + 1):
                nv = mt // 4
                off = (mt % 4) * 128
                ps = ps2.tile([128, CL], f32, tag="ps_v", name="ps_v")
                for k in range(KT):
                    nc.tensor.matmul(
                        ps, x_t[(nv, k)][:, off:off + 128],
                        wv_t[k], start=(k == 0), stop=(k == KT - 1),
                    )
                nc.scalar.copy(out=v_t[mt], in_=ps)
        rtmp.release()
        wvpool.release()
        xpool.release()
        tpool.release()
        ps2.release()
        psr.release()

        # ---- stage D: attention (I outer, h inner), one-tile software
        # pipeline: the PE stream is S0,S1,AV0,d0,S2,AV1,d1,... so exp[j]
        # (ACT) runs while the PE does S[j+1]. Stage E (proj) for chunk I
        # follows its 4 heads, keeping ACT/DVE load smooth.
        pp_pool = tc.alloc_tile_pool(name="pp", bufs=5)
        sm_pool = tc.alloc_tile_pool(name="small", bufs=2)
        ps_s = tc.alloc_tile_pool(name="ps_s", bufs=2, space="PSUM")
        ps_y = tc.alloc_tile_pool(name="ps_y", bufs=2, space="PSUM")
        ps_d = tc.alloc_tile_pool(name="ps_d", bufs=2, space="PSUM")
        ps_o = tc.alloc_tile_pool(name="ps_o", bufs=2, space="PSUM")
        opool = tc.alloc_tile_pool(name="ostage", bufs=3)

        for I in range(NT):
            isl = slice(I * TC, (I + 1) * TC)
            jl = _JLISTS[I]
            nj = len(jl)
            for h in range(HPC):
                q_h = rope[h]
                k_h = rope[4 + h]
                y_ps = ps_y.tile([128, TC], f32, tag="y", name="y_ps")
                d_ps = ps_d.tile([128, TC], f32, tag="d", name="d_ps")

                def emit_S(jidx):
                    J, dm = jl[jidx]
                    lo = 0 if dm is None else 128 * dm
                    csl = slice(lo, TC)
                    s_ps = ps_s.tile([128, TC], f32, tag="s", name="s_ps")
                    nc.tensor.matmul(
                        s_ps[:, csl], k_h[:, J * 128:(J + 1) * 128],
                        q_h[:, I * TC + lo:(I + 1) * TC], start=True, stop=True,
                    )
                    pp = pp_pool.tile([128, TC], bf16, tag="pp", name="pp")
                    nc.scalar.activation(out=pp[:, csl], in_=s_ps[:, csl],
                                         func=Exp, scale=SCALE)
                    return pp

                pps = [None] * nj
                pps[0] = emit_S(0)
                for jidx, (J, dm) in enumerate(jl):
                    if jidx + 1 < nj:
                        pps[jidx + 1] = emit_S(jidx + 1)
                    pp = pps[jidx]
                    # live query-column range of this tile (diagonal tiles
                    # with pattern dm only touch columns >= 128*dm)
                    lo = 0 if dm is None else 128 * dm
                    csl = slice(lo, TC)
                    if dm is not None:
                        ppm = pp_pool.tile([128, TC], bf16, tag="ppm",
                                           name="ppm", bufs=2)
                        nc.vector.tensor_mul(ppm[:, csl], pp[:, csl],
                                             mask_sb[:, dm, csl])
                        pp = ppm
                    first = jidx == 0
                    last = jidx == nj - 1
                    nc.tensor.matmul(
                        y_ps[:, csl], v_t[J][:, h * 128:(h + 1) * 128],
                        pp[:, csl], start=first, stop=last,
                    )
                    nc.tensor.matmul(d_ps[:, csl], ones_sb, pp[:, csl],
                                     start=first, stop=last)
                recip = sm_pool.tile([128, TC], f32, tag="recip", name="recip")
                nc.vector.reciprocal_approx_fast(out=recip, in_=d_ps)
                nc.vector.tensor_mul(yT[h][:, isl], y_ps, recip)

            # ---- stage E for chunk I: out rows = yT^T @ Wp ----
            for ml in range(4):
                mt = 4 * I + ml
                msl = slice(mt * 128, (mt + 1) * 128)
                for n in range(NT):
                    ps = ps_o.tile([128, TC], f32, tag="o", name="o_ps")
                    for hk in range(HPC):
                        nc.tensor.matmul(
                            ps, yT[hk][:, msl], wp_t[hk][:, n * TC:(n + 1) * TC],
                            start=(hk == 0), stop=(hk == HPC - 1),
                        )
                    ot = opool.tile([128, TC], f32, tag="ot", name="ot")
                    nc.vector.tensor_copy(out=ot, in_=ps)
                    nc.sync.dma_start(out=out[msl, n * TC:(n + 1) * TC], in_=ot)

        for p in (opool, sm_pool, pp_pool, v_pool, wppool, y_pool,
                  qk_pool, mpool, ps_o, ps_d, ps_y, ps_s):
            p.release()
    nc.compile()
    return nc


def _host_prep(x, w_qkv, w_proj, freqs_cis):
    """Build per-core input maps (slicing + layout prep only)."""
    import ml_dtypes
    bf16 = ml_dtypes.bfloat16

    x = np.asarray(x, dtype=np.float32)
    w_qkv = np.asarray(w_qkv, dtype=np.float32)
    w_proj = np.asarray(w_proj, dtype=np.float32)
    fc = np.asarray(freqs_cis, dtype=np.float32)

    xTb = [np.ascontiguousarray(x[b].T).astype(bf16) for b in range(B)]

    cos = fc[:, :, 0].T  # [64, T]
    sin = fc[:, :, 1].T
    cosP = np.repeat(cos, 2, axis=0).astype(bf16)  # [128, T]
    sinP = np.repeat(sin, 2, axis=0).astype(bf16)

    rt = np.zeros((HD, HD), dtype=np.float32)
    for d in range(HD // 2):
        rt[2 * d, 2 * d + 1] = 1.0
        rt[2 * d + 1, 2 * d] = -1.0
    rt = rt.astype(bf16)

    masks = np.zeros((4, 128, TC), dtype=np.float32)
    ii = np.arange(TC)[None, :]
    jj = np.arange(128)[:, None]
    for d in range(4):
        masks[d] = (ii >= jj + 128 * d).astype(np.float32)
    masks = masks.astype(bf16)

    ones = np.ones((128, 128), dtype=bf16)

    in_maps = []
    for core in range(8):
        b = core // 4
        g = core % 4
        qc = w_qkv[:, 512 * g: 512 * (g + 1)]
        kc = w_qkv[:, 2048 + 512 * g: 2048 + 512 * (g + 1)]
        vc = np.ascontiguousarray(w_qkv[:, 4096 + 512 * g: 4096 + 512 * (g + 1)]).astype(bf16)
        wqk_c = np.concatenate([qc, kc], axis=1).astype(bf16)
        wp_c = np.ascontiguousarray(w_proj[512 * g: 512 * (g + 1), :]).astype(bf16)
        in_maps.append({
            "xT": xTb[b],
            "wqk": wqk_c,
            "wv": vc,
            "wp": wp_c,
            "cosP": cosP,
            "sinP": sinP,
            "rt": rt,
            "masks": masks,
            "ones": ones,
        })
    return in_maps


def _get_nc():
    if "nc" not in _CACHE:
        _CACHE["nc"] = _build_nc()
    return _CACHE["nc"]


def kernel(x, w_qkv, w_proj, freqs_cis, attn_mask, _trace=False):
    from concourse.bass_utils import run_bass_kernel_spmd

    in_maps = _host_prep(x, w_qkv, w_proj, freqs_cis)
    nc = _get_nc()
    res = run_bass_kernel_spmd(
        nc, in_maps, core_ids=list(range(8)), trace=_trace,
    )
    outs = [r["out"].astype(np.float64) for r in res.results]
    full = np.stack([
        outs[0] + outs[1] + outs[2] + outs[3],
        outs[4] + outs[5] + outs[6] + outs[7],
    ]).astype(np.float32)
    if _trace:
        kernel._last_results = res
    return full


# revision 12
# speedup vs baseline: 1.4076x; 1.0070x over previous
"""Trainium2 Bass kernel for prefix-LM CausalSelfAttention.

Problem: B=2, T=2048, C=2048, H=16 heads (hd=128), prefix-LM mask
(bidirectional over first half, causal after), RoPE on q/k.

Sharding over 8 cores: data-parallel on batch (2) x tensor-parallel on
heads (4 heads per core). Each core computes a partial output projection
(its heads' contribution); partials are summed on host.

All matmul operands are bf16 (PE full rate, f32 PSUM accumulation);
only the final output DMA is f32. Per-core dataflow:
  1. qT/kT = W^T @ x^T    [hd*4, T] head-major tiles; x loaded once
     (bf16) and kept resident for both this and the v matmuls.
  2. RoPE (pair-swap matmul + DVE combine) interleaved with
     v = x @ Wv so the PE never idles while the DVE ropes.
  3. Attention per (chunk I, head h), software-pipelined one key-tile
     ahead so exp (ACT) hides under the PE matmuls:
       S'[J] = k[:,J]^T x q[:,I]   (scores transposed, [j,i])
       P'[J] = exp(S' * scale)     (ACT, psum->sbuf bf16)
       mask-multiply for diagonal-crossing tiles (4 static patterns)
       y_psum += v[J,h]^T x P'[J]
       d_psum += ones128^T x P'[J]  -> full [128,512] broadcast rowsum
     yT[:, I] = y_psum * reciprocal_approx_fast(d_psum)
  4. After each chunk I's 4 heads: partial out rows = yT^T @ Wp.

Fully-masked key tiles are skipped (structural sparsity: 44/64 tiles).
"""
import math

import numpy as np

N_HEAD = 16
B = 2
T = 2048
C = 2048
HD = 128
HPC = 4          # heads per core
CL = HPC * HD    # local C = 512
TC = 512         # chunk width (matmul moving free dim / psum bank)
NT = T // TC     # 4 chunks
KT = C // 128    # 16 contraction tiles over C
TT = T // 128    # 16 T tiles
SCALE = 1.0 / math.sqrt(HD)

# Per query-chunk I: list of (J, mask_idx) key tiles to compute.
# mask_idx is None for fully-allowed tiles, else 0..3 selecting the
# static diagonal pattern mask[d][jj, ii] = (ii >= jj + 128*d).
_JLISTS = {
    0: [(j, None) for j in range(8)],
    1: [(j, None) for j in range(8)],
    2: [(j, None) for j in range(8)] + [(8 + d, d) for d in range(4)],
    3: [(j, None) for j in range(12)] + [(12 + d, d) for d in range(4)],
}

_CACHE = {}


def _build_nc():
    import concourse.tile as tile
    import concourse.mybir as mybir
    from concourse import bacc

    f32 = mybir.dt.float32
    bf16 = mybir.dt.bfloat16

    nc = bacc.Bacc(None, target_bir_lowering=False)

    xT = nc.dram_tensor("xT", [C, T], bf16, kind="ExternalInput")
    wqk = nc.dram_tensor("wqk", [C, 2 * CL], bf16, kind="ExternalInput")
    wv = nc.dram_tensor("wv", [C, CL], bf16, kind="ExternalInput")
    wp = nc.dram_tensor("wp", [CL, C], bf16, kind="ExternalInput")
    cosP = nc.dram_tensor("cosP", [HD, T], bf16, kind="ExternalInput")
    sinP = nc.dram_tensor("sinP", [HD, T], bf16, kind="ExternalInput")
    rt = nc.dram_tensor("rt", [HD, HD], bf16, kind="ExternalInput")
    masks = nc.dram_tensor("masks", [4, 128, TC], bf16, kind="ExternalInput")
    ones = nc.dram_tensor("ones", [128, 128], bf16, kind="ExternalInput")
    out = nc.dram_tensor("out", [T, C], bf16, kind="ExternalOutput")

    xT3 = xT.rearrange("(kt p) t -> p kt t", p=128)
    wqk3 = wqk.rearrange("(kt p) m -> p kt m", p=128)
    wv3 = wv.rearrange("(kt p) m -> p kt m", p=128)
    wp3 = wp.rearrange("(kt p) m -> p kt m", p=128)
    masks3 = masks.rearrange("d p n -> p d n")

    Exp = mybir.ActivationFunctionType.Exp

    with tile.TileContext(nc) as tc:
        mpool = tc.alloc_tile_pool(name="misc", bufs=1)
        qk_pool = tc.alloc_tile_pool(name="qkrope", bufs=1)
        tpool = tc.alloc_tile_pool(name="trig", bufs=1, side="right")

        # PE clock warm-up: ~20 dummy matmuls with no DMA deps keep the
        # tensor engine busy (HAM ramps to full clock) while the first
        # weight/x DMAs land.
        warm = mpool.tile([128, 640], bf16, name="warm")
        nc.vector.memset(warm, 0.0)
        ps_w = tc.alloc_tile_pool(name="ps_warm", bufs=1, space="PSUM")
        wps = ps_w.tile([128, TC], f32, tag="wps", name="wps")
        for _ in range(20):
            nc.tensor.matmul(wps, warm[:, :128], warm[:, 128:640],
                             start=True, stop=True)
        ps_w.release()

        rt_sb = mpool.tile([HD, HD], bf16)
        ones_sb = mpool.tile([128, 128], bf16)
        mask_sb = mpool.tile([128, 4, TC], bf16)
        cos_sb = tpool.tile([HD, T], bf16)
        sin_sb = tpool.tile([HD, T], bf16)

        # qkT[m] for m in 0..7: m<4 -> q head m, else k head m-4; [hd, T]
        # (rope outputs later reuse the same slots via identical tags)
        qkT = [qk_pool.tile([128, T], bf16, tag=f"qk{m}", name=f"qk{m}") for m in range(8)]

        # Long-lived pools first (pool release must be LIFO per side):
        # yT / Wp / v live to the end; x / wv / wqk release after stage C.
        y_pool = tc.alloc_tile_pool(name="yT_sb", bufs=1)          # 16K
        yT = [y_pool.tile([128, T], bf16, tag=f"yT{h}", name=f"yT{h}")
              for h in range(HPC)]
        wppool = tc.alloc_tile_pool(name="wp_sb", bufs=1)          # 16K
        v_pool = tc.alloc_tile_pool(name="v_sb", bufs=1)           # 16K ..attn
        v_t = [v_pool.tile([128, CL], bf16, tag=f"v{mt}", name=f"v{mt}")
               for mt in range(TT)]

        # ---- stage A: qT/kT = W_{q,k}^T @ x^T, head-major tiles ----
        # x (bf16) is loaded ONCE, fully resident; stage C reuses it.
        xpool = tc.alloc_tile_pool(name="xt_all", bufs=1)          # 64K A..C
        wvpool = tc.alloc_tile_pool(name="wv_sb", bufs=1)          # 16K ..v
        wpool = tc.alloc_tile_pool(name="wqk_sb", bufs=1)          # 32K A
        ps1 = tc.alloc_tile_pool(name="ps_qk", bufs=4, space="PSUM")
        w_t = []
        x_t = {}
        for k in range(KT):
            wt = wpool.tile([128, 2 * CL], bf16, tag=f"w{k}", name=f"w{k}")
            nc.sync.dma_start(out=wt, in_=wqk3[:, k])
            w_t.append(wt)
            xt = xpool.tile([128, TC], bf16, tag=f"x0_{k}", name=f"x0_{k}")
            nc.sync.dma_start(out=xt, in_=xT3[:, k, 0:TC])
            x_t[(0, k)] = xt
        nc.sync.dma_start(out=rt_sb, in_=rt[:, :])
        nc.sync.dma_start(out=cos_sb, in_=cosP[:, :])
        nc.sync.dma_start(out=sin_sb, in_=sinP[:, :])
        for n in range(1, NT):
            for k in range(KT):
                xt = xpool.tile([128, TC], bf16, tag=f"x{n}_{k}", name=f"x{n}_{k}")
                nc.sync.dma_start(out=xt, in_=xT3[:, k, n * TC:(n + 1) * TC])
                x_t[(n, k)] = xt

        # v-phase + attention constants DMA'd early (all overlap stage A)
        wv_t = []
        for k in range(KT):
            wt = wvpool.tile([128, CL], bf16, tag=f"wv{k}", name=f"wv{k}")
            nc.sync.dma_start(out=wt, in_=wv3[:, k])
            wv_t.append(wt)
        nc.sync.dma_start(out=ones_sb, in_=ones[:, :])
        nc.sync.dma_start(out=mask_sb, in_=masks3)
        wp_t = []
        for hk in range(HPC):
            wt = wppool.tile([128, C], bf16, tag=f"wp{hk}", name=f"wp{hk}")
            nc.sync.dma_start(out=wt, in_=wp3[:, hk])
            wp_t.append(wt)

        for n in range(NT):
            for m in range(8):
                ps = ps1.tile([128, TC], f32, tag="ps_qk", name="ps_qk")
                for k in range(KT):
                    nc.tensor.matmul(
                        ps, w_t[k][:, m * 128:(m + 1) * 128], x_t[(n, k)],
                        start=(k == 0), stop=(k == KT - 1),
                    )
                nc.vector.tensor_copy(out=qkT[m][:, n * TC:(n + 1) * TC], in_=ps)
        wpool.release()
        ps1.release()

        # ---- stage B+C interleaved: RoPE (DVE-heavy) + v = x @ Wv
        # (PE-heavy) so the PE never waits on the DVE rope chain.
        # rope = qkT*cos + (R @ qkT)*sin ; R = pair swap w/ sign.
        rtmp = tc.alloc_tile_pool(name="rope_tmp", bufs=4)
        psr = tc.alloc_tile_pool(name="ps_rot", bufs=4, space="PSUM")
        ps2 = tc.alloc_tile_pool(name="ps_v", bufs=4, space="PSUM")
        rope = [None] * 8
        for m in range(8):
            tmp = []
            for n in range(NT):
                sl = slice(n * TC, (n + 1) * TC)
                ps = psr.tile([128, TC], f32, tag="ps_rot", name="ps_rot")
                nc.tensor.matmul(ps, rt_sb, qkT[m][:, sl], start=True, stop=True)
                t1 = rtmp.tile([128, TC], bf16, tag="t1", name="t1")
                t2 = rtmp.tile([128, TC], bf16, tag="t2", name="t2")
                nc.vector.tensor_mul(t1, ps, sin_sb[:, sl])
                nc.vector.tensor_mul(t2, qkT[m][:, sl], cos_sb[:, sl])
                tmp.append((t1, t2))
            # all reads of qkT[m] issued; now write into its slot
            ro = qk_pool.tile([128, T], bf16, tag=f"qk{m}", name=f"rope{m}")
            for n in range(NT):
                sl = slice(n * TC, (n + 1) * TC)
                nc.vector.tensor_add(ro[:, sl], tmp[n][0], tmp[n][1])
            rope[m] = ro
            # two v T-tiles per rope head: PE work covering the DVE chain
            for mt in (2 * m, 2 * m + 1):
                nv = mt // 4
                off = (mt % 4) * 128
                ps = ps2.tile([128, CL], f32, tag="ps_v", name="ps_v")
                for k in range(KT):
                    nc.tensor.matmul(
                        ps, x_t[(nv, k)][:, off:off + 128],
                        wv_t[k], start=(k == 0), stop=(k == KT - 1),
                    )
                nc.scalar.copy(out=v_t[mt], in_=ps)
        rtmp.release()
        wvpool.release()
        xpool.release()
        tpool.release()
        ps2.release()
        psr.release()

        # ---- stage D: attention (I outer, h inner), one-tile software
        # pipeline: the PE stream is S0,S1,AV0,d0,S2,AV1,d1,... so exp[j]
        # (ACT) runs while the PE does S[j+1]. Stage E (proj) for chunk I
        # follows its 4 heads, keeping ACT/DVE load smooth.
        pp_pool = tc.alloc_tile_pool(name="pp", bufs=5)
        sm_pool = tc.alloc_tile_pool(name="small", bufs=2)
        ps_s = tc.alloc_tile_pool(name="ps_s", bufs=2, space="PSUM")
        ps_y = tc.alloc_tile_pool(name="ps_y", bufs=2, space="PSUM")
        ps_d = tc.alloc_tile_pool(name="ps_d", bufs=2, space="PSUM")
        ps_o = tc.alloc_tile_pool(name="ps_o", bufs=2, space="PSUM")
        opool = tc.alloc_tile_pool(name="ostage", bufs=3)

        for I in range(NT):
            isl = slice(I * TC, (I + 1) * TC)
            jl = _JLISTS[I]
            nj = len(jl)
            for h in range(HPC):
                q_h = rope[h]
                k_h = rope[4 + h]
                y_ps = ps_y.tile([128, TC], f32, tag="y", name="y_ps")
                d_ps = ps_d.tile([128, TC], f32, tag="d", name="d_ps")

                def emit_S(jidx):
                    J, dm = jl[jidx]
                    lo = 0 if dm is None else 128 * dm
                    csl = slice(lo, TC)
                    s_ps = ps_s.tile([128, TC], f32, tag="s", name="s_ps")
                    nc.tensor.matmul(
                        s_ps[:, csl], k_h[:, J * 128:(J + 1) * 128],
                        q_h[:, I * TC + lo:(I + 1) * TC], start=True, stop=True,
                    )
                    pp = pp_pool.tile([128, TC], bf16, tag="pp", name="pp")
                    nc.scalar.activation(out=pp[:, csl], in_=s_ps[:, csl],
                                         func=Exp, scale=SCALE)
                    return pp

                pps = [None] * nj
                pps[0] = emit_S(0)
                for jidx, (J, dm) in enumerate(jl):
                    if jidx + 1 < nj:
                        pps[jidx + 1] = emit_S(jidx + 1)
                    pp = pps[jidx]
                    # live query-column range of this tile (diagonal tiles
                    # with pattern dm only touch columns >= 128*dm)
                    lo = 0 if dm is None else 128 * dm
                    csl = slice(lo, TC)
                    if dm is not None:
                        ppm = pp_pool.tile([128, TC], bf16, tag="ppm",
                                           name="ppm", bufs=2)
                        nc.vector.tensor_mul(ppm[:, csl], pp[:, csl],
                                             mask_sb[:, dm, csl])
                        pp = ppm
                    first = jidx == 0
                    last = jidx == nj - 1
                    nc.tensor.matmul(
                        y_ps[:, csl], v_t[J][:, h * 128:(h + 1) * 128],
                        pp[:, csl], start=first, stop=last,
                    )
                    nc.tensor.matmul(d_ps[:, csl], ones_sb, pp[:, csl],
                                     start=first, stop=last)
                recip = sm_pool.tile([128, TC], f32, tag="recip", name="recip")
                nc.vector.reciprocal_approx_fast(out=recip, in_=d_ps)
                nc.vector.tensor_mul(yT[h][:, isl], y_ps, recip)

            # ---- stage E for chunk I: out rows = yT^T @ Wp ----
            for ml in range(4):
                mt = 4 * I + ml
                msl = slice(mt * 128, (mt + 1) * 128)
                for n in range(NT):
                    ps = ps_o.tile([128, TC], f32, tag="o", name="o_ps")
                    for hk in range(HPC):
                        nc.tensor.matmul(
                            ps, yT[hk][:, msl], wp_t[hk][:, n * TC:(n + 1) * TC],
                            start=(hk == 0), stop=(hk == HPC - 1),
                        )
                    ot = opool.tile([128, TC], bf16, tag="ot", name="ot")
                    if n % 2 == 0:
                        nc.vector.tensor_copy(out=ot, in_=ps)
                    else:
                        nc.scalar.copy(out=ot, in_=ps)
                    nc.sync.dma_start(out=out[msl, n * TC:(n + 1) * TC], in_=ot)

        for p in (opool, sm_pool, pp_pool, v_pool, wppool, y_pool,
                  qk_pool, mpool, ps_o, ps_d, ps_y, ps_s):
            p.release()
    nc.compile()
    return nc


def _host_prep(x, w_qkv, w_proj, freqs_cis):
    """Build per-core input maps (slicing + layout prep only)."""
    import ml_dtypes
    bf16 = ml_dtypes.bfloat16

    x = np.asarray(x, dtype=np.float32)
    w_qkv = np.asarray(w_qkv, dtype=np.float32)
    w_proj = np.asarray(w_proj, dtype=np.float32)
    fc = np.asarray(freqs_cis, dtype=np.float32)

    xTb = [np.ascontiguousarray(x[b].T).astype(bf16) for b in range(B)]

    cos = fc[:, :, 0].T  # [64, T]
    sin = fc[:, :, 1].T
    cosP = np.repeat(cos, 2, axis=0).astype(bf16)  # [128, T]
    sinP = np.repeat(sin, 2, axis=0).astype(bf16)

    rt = np.zeros((HD, HD), dtype=np.float32)
    for d in range(HD // 2):
        rt[2 * d, 2 * d + 1] = 1.0
        rt[2 * d + 1, 2 * d] = -1.0
    rt = rt.astype(bf16)

    masks = np.zeros((4, 128, TC), dtype=np.float32)
    ii = np.arange(TC)[None, :]
    jj = np.arange(128)[:, None]
    for d in range(4):
        masks[d] = (ii >= jj + 128 * d).astype(np.float32)
    masks = masks.astype(bf16)

    ones = np.ones((128, 128), dtype=bf16)

    in_maps = []
    for core in range(8):
        b = core // 4
        g = core % 4
        qc = w_qkv[:, 512 * g: 512 * (g + 1)]
        kc = w_qkv[:, 2048 + 512 * g: 2048 + 512 * (g + 1)]
        vc = np.ascontiguousarray(w_qkv[:, 4096 + 512 * g: 4096 + 512 * (g + 1)]).astype(bf16)
        wqk_c = np.concatenate([qc, kc], axis=1).astype(bf16)
        wp_c = np.ascontiguousarray(w_proj[512 * g: 512 * (g + 1), :]).astype(bf16)
        in_maps.append({
            "xT": xTb[b],
            "wqk": wqk_c,
            "wv": vc,
            "wp": wp_c,
            "cosP": cosP,
            "sinP": sinP,
            "rt": rt,
            "masks": masks,
            "ones": ones,
        })
    return in_maps


def _get_nc():
    if "nc" not in _CACHE:
        _CACHE["nc"] = _build_nc()
    return _CACHE["nc"]


def kernel(x, w_qkv, w_proj, freqs_cis, attn_mask, _trace=False):
    from concourse.bass_utils import run_bass_kernel_spmd

    in_maps = _host_prep(x, w_qkv, w_proj, freqs_cis)
    nc = _get_nc()
    res = run_bass_kernel_spmd(
        nc, in_maps, core_ids=list(range(8)), trace=_trace,
    )
    outs = [r["out"].astype(np.float64) for r in res.results]
    full = np.stack([
        outs[0] + outs[1] + outs[2] + outs[3],
        outs[4] + outs[5] + outs[6] + outs[7],
    ]).astype(np.float32)
    if _trace:
        kernel._last_results = res
    return full


# revision 13
# speedup vs baseline: 1.4655x; 1.0411x over previous
"""Trainium2 Bass kernel for prefix-LM CausalSelfAttention.

Problem: B=2, T=2048, C=2048, H=16 heads (hd=128), prefix-LM mask
(bidirectional over first half, causal after), RoPE on q/k.

Sharding over 8 cores: data-parallel on batch (2) x tensor-parallel on
heads (4 heads per core). Each core computes a partial output projection
(its heads' contribution); partials are summed on host.

All matmul operands are bf16 (PE full rate, f32 PSUM accumulation);
only the final output DMA is f32. Per-core dataflow:
  1. qT/kT = W^T @ x^T    [hd*4, T] head-major tiles; x loaded once
     (bf16) and kept resident for both this and the v matmuls.
  2. RoPE (pair-swap matmul + DVE combine) interleaved with
     v = x @ Wv so the PE never idles while the DVE ropes.
  3. Attention per (chunk I, head h), software-pipelined one key-tile
     ahead so exp (ACT) hides under the PE matmuls:
       S'[J] = k[:,J]^T x q[:,I]   (scores transposed, [j,i])
       P'[J] = exp(S' * scale)     (ACT, psum->sbuf bf16)
       mask-multiply for diagonal-crossing tiles (4 static patterns)
       y_psum += v[J,h]^T x P'[J]
       d_psum += ones128^T x P'[J]  -> full [128,512] broadcast rowsum
     yT[:, I] = y_psum * reciprocal_approx_fast(d_psum)
  4. After each chunk I's 4 heads: partial out rows = yT^T @ Wp.

Fully-masked key tiles are skipped (structural sparsity: 44/64 tiles).
"""
import math

import numpy as np

N_HEAD = 16
B = 2
T = 2048
C = 2048
HD = 128
HPC = 4          # heads per core
CL = HPC * HD    # local C = 512
TC = 512         # chunk width (matmul moving free dim / psum bank)
NT = T // TC     # 4 chunks
KT = C // 128    # 16 contraction tiles over C
TT = T // 128    # 16 T tiles
SCALE = 1.0 / math.sqrt(HD)

# Per query-chunk I: list of (J, mask_idx) key tiles to compute.
# mask_idx is None for fully-allowed tiles, else 0..3 selecting the
# static diagonal pattern mask[d][jj, ii] = (ii >= jj + 128*d).
_JLISTS = {
    0: [(j, None) for j in range(8)],
    1: [(j, None) for j in range(8)],
    2: [(j, None) for j in range(8)] + [(8 + d, d) for d in range(4)],
    3: [(j, None) for j in range(12)] + [(12 + d, d) for d in range(4)],
}

_CACHE = {}


def _build_nc():
    import concourse.tile as tile
    import concourse.mybir as mybir
    from concourse import bacc

    f32 = mybir.dt.float32
    bf16 = mybir.dt.bfloat16

    nc = bacc.Bacc(None, target_bir_lowering=False)

    xT = nc.dram_tensor("xT", [C, T], bf16, kind="ExternalInput")
    wqk = nc.dram_tensor("wqk", [C, 2 * CL], bf16, kind="ExternalInput")
    wv = nc.dram_tensor("wv", [C, CL], bf16, kind="ExternalInput")
    wp = nc.dram_tensor("wp", [CL, C], bf16, kind="ExternalInput")
    cosP = nc.dram_tensor("cosP", [HD, T], bf16, kind="ExternalInput")
    sinP = nc.dram_tensor("sinP", [HD, T], bf16, kind="ExternalInput")
    rt = nc.dram_tensor("rt", [HD, HD], bf16, kind="ExternalInput")
    masks = nc.dram_tensor("masks", [4, 128, TC], bf16, kind="ExternalInput")
    ones = nc.dram_tensor("ones", [128, 128], bf16, kind="ExternalInput")
    out = nc.dram_tensor("out", [T, C], bf16, kind="ExternalOutput")

    xT3 = xT.rearrange("(kt p) t -> p kt t", p=128)
    wqk3 = wqk.rearrange("(kt p) m -> p kt m", p=128)
    wv3 = wv.rearrange("(kt p) m -> p kt m", p=128)
    wp3 = wp.rearrange("(kt p) m -> p kt m", p=128)
    masks3 = masks.rearrange("d p n -> p d n")

    Exp = mybir.ActivationFunctionType.Exp

    with tile.TileContext(nc) as tc:
        mpool = tc.alloc_tile_pool(name="misc", bufs=1)
        qk_pool = tc.alloc_tile_pool(name="qkrope", bufs=1)
        tpool = tc.alloc_tile_pool(name="trig", bufs=1, side="right")

        # PE clock warm-up: ~20 dummy matmuls with no DMA deps keep the
        # tensor engine busy (HAM ramps to full clock) while the first
        # weight/x DMAs land.
        warm = mpool.tile([128, 640], bf16, name="warm")
        nc.vector.memset(warm, 0.0)
        ps_w = tc.alloc_tile_pool(name="ps_warm", bufs=1, space="PSUM")
        wps = ps_w.tile([128, TC], f32, tag="wps", name="wps")
        for _ in range(20):
            nc.tensor.matmul(wps, warm[:, :128], warm[:, 128:640],
                             start=True, stop=True)
        ps_w.release()

        rt_sb = mpool.tile([HD, HD], bf16)
        ones_sb = mpool.tile([128, 128], bf16)
        mask_sb = mpool.tile([128, 4, TC], bf16)
        cos_sb = tpool.tile([HD, T], bf16)
        sin_sb = tpool.tile([HD, T], bf16)

        # qkT[m] for m in 0..7: m<4 -> q head m, else k head m-4; [hd, T]
        # (rope outputs later reuse the same slots via identical tags)
        qkT = [qk_pool.tile([128, T], bf16, tag=f"qk{m}", name=f"qk{m}") for m in range(8)]

        # Long-lived pools first (pool release must be LIFO per side):
        # yT / Wp / v live to the end; x / wv / wqk release after stage C.
        y_pool = tc.alloc_tile_pool(name="yT_sb", bufs=1)          # 16K
        yT = [y_pool.tile([128, T], bf16, tag=f"yT{h}", name=f"yT{h}")
              for h in range(HPC)]
        wppool = tc.alloc_tile_pool(name="wp_sb", bufs=1)          # 16K
        v_pool = tc.alloc_tile_pool(name="v_sb", bufs=1)           # 16K ..attn
        v_t = [v_pool.tile([128, CL], bf16, tag=f"v{mt}", name=f"v{mt}")
               for mt in range(TT)]

        # ---- stage A: qT/kT = W_{q,k}^T @ x^T, head-major tiles ----
        # x (bf16) is loaded ONCE, fully resident; stage C reuses it.
        xpool = tc.alloc_tile_pool(name="xt_all", bufs=1)          # 64K A..C
        wvpool = tc.alloc_tile_pool(name="wv_sb", bufs=1)          # 16K ..v
        wpool = tc.alloc_tile_pool(name="wqk_sb", bufs=1)          # 32K A
        ps1 = tc.alloc_tile_pool(name="ps_qk", bufs=4, space="PSUM")
        w_t = []
        x_t = {}
        for k in range(KT):
            wt = wpool.tile([128, 2 * CL], bf16, tag=f"w{k}", name=f"w{k}")
            nc.sync.dma_start(out=wt, in_=wqk3[:, k])
            w_t.append(wt)
            xt = xpool.tile([128, TC], bf16, tag=f"x0_{k}", name=f"x0_{k}")
            nc.scalar.dma_start(out=xt, in_=xT3[:, k, 0:TC])
            x_t[(0, k)] = xt
        nc.sync.dma_start(out=rt_sb, in_=rt[:, :])
        nc.sync.dma_start(out=cos_sb, in_=cosP[:, :])
        nc.sync.dma_start(out=sin_sb, in_=sinP[:, :])
        for n in range(1, NT):
            for k in range(KT):
                xt = xpool.tile([128, TC], bf16, tag=f"x{n}_{k}", name=f"x{n}_{k}")
                nc.scalar.dma_start(out=xt, in_=xT3[:, k, n * TC:(n + 1) * TC])
                x_t[(n, k)] = xt

        # v-phase + attention constants DMA'd early (all overlap stage A)
        wv_t = []
        for k in range(KT):
            wt = wvpool.tile([128, CL], bf16, tag=f"wv{k}", name=f"wv{k}")
            nc.sync.dma_start(out=wt, in_=wv3[:, k])
            wv_t.append(wt)
        nc.sync.dma_start(out=ones_sb, in_=ones[:, :])
        nc.sync.dma_start(out=mask_sb, in_=masks3)
        wp_t = []
        for hk in range(HPC):
            wt = wppool.tile([128, C], bf16, tag=f"wp{hk}", name=f"wp{hk}")
            nc.sync.dma_start(out=wt, in_=wp3[:, hk])
            wp_t.append(wt)

        for n in range(NT):
            for m in range(8):
                ps = ps1.tile([128, TC], f32, tag="ps_qk", name="ps_qk")
                for k in range(KT):
                    nc.tensor.matmul(
                        ps, w_t[k][:, m * 128:(m + 1) * 128], x_t[(n, k)],
                        start=(k == 0), stop=(k == KT - 1),
                    )
                nc.vector.tensor_copy(out=qkT[m][:, n * TC:(n + 1) * TC], in_=ps)
        wpool.release()
        ps1.release()

        # ---- stage B+C interleaved: RoPE (DVE-heavy) + v = x @ Wv
        # (PE-heavy) so the PE never waits on the DVE rope chain.
        # rope = qkT*cos + (R @ qkT)*sin ; R = pair swap w/ sign.
        rtmp = tc.alloc_tile_pool(name="rope_tmp", bufs=4)
        psr = tc.alloc_tile_pool(name="ps_rot", bufs=4, space="PSUM")
        ps2 = tc.alloc_tile_pool(name="ps_v", bufs=4, space="PSUM")
        rope = [None] * 8
        for m in range(8):
            tmp = []
            for n in range(NT):
                sl = slice(n * TC, (n + 1) * TC)
                ps = psr.tile([128, TC], f32, tag="ps_rot", name="ps_rot")
                nc.tensor.matmul(ps, rt_sb, qkT[m][:, sl], start=True, stop=True)
                t1 = rtmp.tile([128, TC], bf16, tag="t1", name="t1")
                t2 = rtmp.tile([128, TC], bf16, tag="t2", name="t2")
                nc.vector.tensor_mul(t1, ps, sin_sb[:, sl])
                nc.vector.tensor_mul(t2, qkT[m][:, sl], cos_sb[:, sl])
                tmp.append((t1, t2))
            # all reads of qkT[m] issued; now write into its slot
            ro = qk_pool.tile([128, T], bf16, tag=f"qk{m}", name=f"rope{m}")
            for n in range(NT):
                sl = slice(n * TC, (n + 1) * TC)
                nc.vector.tensor_add(ro[:, sl], tmp[n][0], tmp[n][1])
            rope[m] = ro
            # two v T-tiles per rope head: PE work covering the DVE chain
            for mt in (2 * m, 2 * m + 1):
                nv = mt // 4
                off = (mt % 4) * 128
                ps = ps2.tile([128, CL], f32, tag="ps_v", name="ps_v")
                for k in range(KT):
                    nc.tensor.matmul(
                        ps, x_t[(nv, k)][:, off:off + 128],
                        wv_t[k], start=(k == 0), stop=(k == KT - 1),
                    )
                nc.scalar.copy(out=v_t[mt], in_=ps)
        rtmp.release()
        wvpool.release()
        xpool.release()
        tpool.release()
        ps2.release()
        psr.release()

        # ---- stage D: attention (I outer, h inner), one-tile software
        # pipeline: the PE stream is S0,S1,AV0,d0,S2,AV1,d1,... so exp[j]
        # (ACT) runs while the PE does S[j+1]. Stage E (proj) for chunk I
        # follows its 4 heads, keeping ACT/DVE load smooth.
        pp_pool = tc.alloc_tile_pool(name="pp", bufs=5)
        sm_pool = tc.alloc_tile_pool(name="small", bufs=2)
        ps_s = tc.alloc_tile_pool(name="ps_s", bufs=2, space="PSUM")
        ps_y = tc.alloc_tile_pool(name="ps_y", bufs=2, space="PSUM")
        ps_d = tc.alloc_tile_pool(name="ps_d", bufs=2, space="PSUM")
        ps_o = tc.alloc_tile_pool(name="ps_o", bufs=2, space="PSUM")
        opool = tc.alloc_tile_pool(name="ostage", bufs=3)

        def emit_proj(Ip, ml):
            # one 128-row out block: out[mt] = sum_h yT[h][:,rows]^T @ Wp[h]
            mt = 4 * Ip + ml
            msl = slice(mt * 128, (mt + 1) * 128)
            ot = opool.tile([128, C], bf16, tag="ot", name="ot")
            for n in range(NT):
                ps = ps_o.tile([128, TC], f32, tag="o", name="o_ps")
                for hk in range(HPC):
                    nc.tensor.matmul(
                        ps, yT[hk][:, msl], wp_t[hk][:, n * TC:(n + 1) * TC],
                        start=(hk == 0), stop=(hk == HPC - 1),
                    )
                osl = slice(n * TC, (n + 1) * TC)
                if n % 2 == 0:
                    nc.vector.tensor_copy(out=ot[:, osl], in_=ps)
                else:
                    nc.scalar.copy(out=ot[:, osl], in_=ps)
            nc.sync.dma_start(out=out[msl, :], in_=ot)

        for I in range(NT):
            isl = slice(I * TC, (I + 1) * TC)
            jl = _JLISTS[I]
            nj = len(jl)
            for h in range(HPC):
                q_h = rope[h]
                k_h = rope[4 + h]
                y_ps = ps_y.tile([128, TC], f32, tag="y", name="y_ps")
                d_ps = ps_d.tile([128, TC], f32, tag="d", name="d_ps")

                def emit_S(jidx):
                    J, dm = jl[jidx]
                    lo = 0 if dm is None else 128 * dm
                    csl = slice(lo, TC)
                    s_ps = ps_s.tile([128, TC], f32, tag="s", name="s_ps")
                    nc.tensor.matmul(
                        s_ps[:, csl], k_h[:, J * 128:(J + 1) * 128],
                        q_h[:, I * TC + lo:(I + 1) * TC], start=True, stop=True,
                    )
                    pp = pp_pool.tile([128, TC], bf16, tag="pp", name="pp")
                    nc.scalar.activation(out=pp[:, csl], in_=s_ps[:, csl],
                                         func=Exp, scale=SCALE)
                    return pp

                pps = [None] * nj
                pps[0] = emit_S(0)
                for jidx, (J, dm) in enumerate(jl):
                    if jidx + 1 < nj:
                        pps[jidx + 1] = emit_S(jidx + 1)
                    pp = pps[jidx]
                    # live query-column range of this tile (diagonal tiles
                    # with pattern dm only touch columns >= 128*dm)
                    lo = 0 if dm is None else 128 * dm
                    csl = slice(lo, TC)
                    if dm is not None:
                        ppm = pp_pool.tile([128, TC], bf16, tag="ppm",
                                           name="ppm", bufs=2)
                        nc.vector.tensor_mul(ppm[:, csl], pp[:, csl],
                                             mask_sb[:, dm, csl])
                        pp = ppm
                    first = jidx == 0
                    last = jidx == nj - 1
                    nc.tensor.matmul(
                        y_ps[:, csl], v_t[J][:, h * 128:(h + 1) * 128],
                        pp[:, csl], start=first, stop=last,
                    )
                    nc.tensor.matmul(d_ps[:, csl], ones_sb, pp[:, csl],
                                     start=first, stop=last)
                recip = sm_pool.tile([128, TC], f32, tag="recip", name="recip")
                nc.vector.reciprocal_approx_fast(out=recip, in_=d_ps)
                nc.vector.tensor_mul(yT[h][:, isl], y_ps, recip)
                # previous chunk's proj row-block: pure-PE work with no
                # ACT dependency, letting the exp queue catch up
                if I > 0:
                    emit_proj(I - 1, h)
        for ml in range(4):
            emit_proj(NT - 1, ml)

        for p in (opool, sm_pool, pp_pool, v_pool, wppool, y_pool,
                  qk_pool, mpool, ps_o, ps_d, ps_y, ps_s):
            p.release()
    nc.compile()
    return nc


def _host_prep(x, w_qkv, w_proj, freqs_cis):
    """Build per-core input maps (slicing + layout prep only)."""
    import ml_dtypes
    bf16 = ml_dtypes.bfloat16

    x = np.asarray(x, dtype=np.float32)
    w_qkv = np.asarray(w_qkv, dtype=np.float32)
    w_proj = np.asarray(w_proj, dtype=np.float32)
    fc = np.asarray(freqs_cis, dtype=np.float32)

    xTb = [np.ascontiguousarray(x[b].T).astype(bf16) for b in range(B)]

    cos = fc[:, :, 0].T  # [64, T]
    sin = fc[:, :, 1].T
    cosP = np.repeat(cos, 2, axis=0).astype(bf16)  # [128, T]
    sinP = np.repeat(sin, 2, axis=0).astype(bf16)

    rt = np.zeros((HD, HD), dtype=np.float32)
    for d in range(HD // 2):
        rt[2 * d, 2 * d + 1] = 1.0
        rt[2 * d + 1, 2 * d] = -1.0
    rt = rt.astype(bf16)

    masks = np.zeros((4, 128, TC), dtype=np.float32)
    ii = np.arange(TC)[None, :]
    jj = np.arange(128)[:, None]
    for d in range(4):
        masks[d] = (ii >= jj + 128 * d).astype(np.float32)
    masks = masks.astype(bf16)

    ones = np.ones((128, 128), dtype=bf16)

    in_maps = []
    for core in range(8):
        b = core // 4
        g = core % 4
        qc = w_qkv[:, 512 * g: 512 * (g + 1)]
        kc = w_qkv[:, 2048 + 512 * g: 2048 + 512 * (g + 1)]
        vc = np.ascontiguousarray(w_qkv[:, 4096 + 512 * g: 4096 + 512 * (g + 1)]).astype(bf16)
        wqk_c = np.concatenate([qc, kc], axis=1).astype(bf16)
        wp_c = np.ascontiguousarray(w_proj[512 * g: 512 * (g + 1), :]).astype(bf16)
        in_maps.append({
            "xT": xTb[b],
            "wqk": wqk_c,
            "wv": vc,
            "wp": wp_c,
            "cosP": cosP,
            "sinP": sinP,
            "rt": rt,
            "masks": masks,
            "ones": ones,
        })
    return in_maps


def _get_nc():
    if "nc" not in _CACHE:
        _CACHE["nc"] = _build_nc()
    return _CACHE["nc"]


def kernel(x, w_qkv, w_proj, freqs_cis, attn_mask, _trace=False):
    from concourse.bass_utils import run_bass_kernel_spmd

    in_maps = _host_prep(x, w_qkv, w_proj, freqs_cis)
    nc = _get_nc()
    res = run_bass_kernel_spmd(
        nc, in_maps, core_ids=list(range(8)), trace=_trace,
    )
    outs = [r["out"].astype(np.float64) for r in res.results]
    full = np.stack([
        outs[0] + outs[1] + outs[2] + outs[3],
        outs[4] + outs[5] + outs[6] + outs[7],
    ]).astype(np.float32)
    if _trace:
        kernel._last_results = res
    return full
